# revision 4
# baseline (speedup 1.0000x reference)
"""Bahdanau attention on 8 Trainium2 NeuronCores.

Data-parallel over batch: each core handles B_L = B/8 = 4 batches with all
weights replicated.  Per batch b:
  keyT[h,s]  = sum_e WkT[e,h] * encT[e,s]         (PE, fp32r)
  T[h,s]     = tanh(keyT + qT[h,b])               (ACT, bias = per-partition)
  scores[s]  = sum_h WeT[h] * T[h,s]              (PE, M=1 matmul)
  alph       = exp(scores - max)                  (ACT, accum_out = sum)
  ctx[e]     = (1/sum) * sum_s alph[s] * enc[s,e] (PE, second pass over enc)

enc arrives [S, E] in HBM; the key matmul needs it E-on-partitions, so each
[128,128] block is transposed on the PE (identity-matmul) and cast to fp32r
on the PSUM->SBUF copy.  The context matmul uses enc in native layout.
"""

import sys

if "/opt/trn_rl_repo" not in sys.path:
    sys.path.insert(0, "/opt/trn_rl_repo")

import numpy as np

import concourse.bass as bass
import concourse.mybir as mybir
from concourse import bacc
from concourse.tile import TileContext
from concourse.bass_utils import run_bass_kernel_spmd
from concourse.masks import make_identity

B, S, H = 32, 2048, 1024
E = 2 * H
N_CORES = 8
B_L = B // N_CORES       # 4 batches per core
SC = 512                 # s-chunk for the key/scores phase
NSC = S // SC            # 4
SB = S // 128            # 16 s-blocks of 128
EC = E // 128            # 16 e-chunks
HC = H // 128            # 8 h-chunks
EQ = E // 512            # 4 e-quarters for the context phase

F32 = mybir.dt.float32
F32R = mybir.dt.float32r
ACT_F = mybir.ActivationFunctionType
AX = mybir.AxisListType

_CACHE = {}


def _build(repeat=1):
    key = ("nc", repeat)
    if key in _CACHE:
        return _CACHE[key]
    nc = bacc.Bacc("TRN2", target_bir_lowering=False, debug=False,
                   num_devices=N_CORES)
    enc = nc.dram_tensor("enc", [B_L, S, E], F32, kind="ExternalInput").ap()
    dec = nc.dram_tensor("dec", [B_L, H], F32, kind="ExternalInput").ap()
    wq = nc.dram_tensor("Wq", [H, H], F32, kind="ExternalInput").ap()
    wk = nc.dram_tensor("Wk", [H, E], F32, kind="ExternalInput").ap()
    we = nc.dram_tensor("We", [1, H], F32, kind="ExternalInput").ap()
    out = nc.dram_tensor("out", [B_L, E], F32, kind="ExternalOutput").ap()

    with TileContext(nc) as tc:
        with (
            tc.tile_pool(name="const", bufs=1) as cpool,
            tc.tile_pool(name="dram", bufs=2, space="DRAM") as dpool,
            tc.tile_pool(name="enc_in", bufs=2) as epool,
            tc.tile_pool(name="encT", bufs=1) as etpool,
            tc.tile_pool(name="tpool", bufs=1) as tpool,
            tc.tile_pool(name="cstage", bufs=4) as cstage,
            tc.tile_pool(name="scores", bufs=2) as scpool,
            tc.tile_pool(name="alph", bufs=1) as apool,
            tc.tile_pool(name="small", bufs=2) as small,
            tc.tile_pool(name="tr_psum", bufs=2, space="PSUM") as tr_psum,
            tc.tile_pool(name="key_psum", bufs=2, space="PSUM") as key_psum,
            tc.tile_pool(name="sc_psum", bufs=2, space="PSUM") as sc_psum,
            tc.tile_pool(name="ctx_psum", bufs=2, space="PSUM") as ctx_psum,
        ):
            # ---------------- setup ----------------
            ident = cpool.tile([128, 128], F32)
            make_identity(nc, ident)

            # WkT[ec] : [128e, H] fp32r, resident all kernel
            wkT = [cpool.tile([128, H], F32R, name=f"wkT{ec}") for ec in range(EC)]
            for hc in range(HC):
                stg = epool.tile([128, 2, E], F32, name="enc_in")
                nc.sync.dma_start(stg[:, 0, :], wk[hc * 128:(hc + 1) * 128, :])
                for ec in range(EC):
                    ps = tr_psum.tile([128, 128], F32, name="tr")
                    nc.tensor.transpose(ps[:], stg[:, 0, ec * 128:(ec + 1) * 128], ident[:])
                    nc.vector.tensor_copy(wkT[ec][:, hc * 128:(hc + 1) * 128], ps[:])

            # qT[h, b] = sum_e WqT[e,h] dec[b,e]   ([128, HC, B_L] fp32 for ACT bias)
            decT = cpool.tile([128, HC, B_L], F32R)
            for b2 in range(B_L):
                nc.sync.dma_start(
                    decT[:, :, b2],
                    dec.bitcast(F32R)[b2, :].rearrange("(c p) -> p c", p=128))
            qT = cpool.tile([128, HC, B_L], F32)
            for hc in range(HC):
                stg = epool.tile([128, 2, E], F32, name="enc_in")
                nc.sync.dma_start(stg[:, 0, :H], wq[hc * 128:(hc + 1) * 128, :])
                qps = key_psum.tile([128, SC], F32, name="key")
                for ec in range(HC):
                    ps = tr_psum.tile([128, 128], F32, name="tr")
                    nc.tensor.transpose(ps[:], stg[:, 0, ec * 128:(ec + 1) * 128], ident[:])
                    blk = small.tile([128, 128], F32R, name="wq_blk")
                    nc.vector.tensor_copy(blk[:], ps[:])
                    nc.tensor.matmul(qps[:, :B_L], blk[:], decT[:, ec, :],
                                     start=(ec == 0), stop=(ec == HC - 1))
                nc.vector.tensor_copy(qT[:, hc, :], qps[:, :B_L])

            # WeT : [128, HC] fp32r
            weT = cpool.tile([128, HC], F32R)
            nc.sync.dma_start(weT[:], we.bitcast(F32R).rearrange("o (c p) -> p (o c)", p=128))

            # persistent working tiles
            encT = [etpool.tile([128, SC], F32R, name=f"encT{ec}") for ec in range(EC)]
            T_sb = [tpool.tile([128, SC], F32R, name=f"T{hc}") for hc in range(HC)]

            def phase_a(b):
                scores = scpool.tile([1, S], F32, name="scores")
                for sc in range(NSC):
                    for half in range(2):
                        est = epool.tile([128, 2, E], F32, name="enc_in")
                        for s2 in range(2):
                            r0 = b, (sc * SC + (half * 2 + s2) * 128)
                            nc.sync.dma_start(
                                est[:, s2, :],
                                enc[b, sc * SC + (half * 2 + s2) * 128:
                                       sc * SC + (half * 2 + s2 + 1) * 128, :])
                        for s2 in range(2):
                            ss = half * 2 + s2
                            for ec in range(EC):
                                ps = tr_psum.tile([128, 128], F32, name="tr")
                                nc.tensor.transpose(
                                    ps[:], est[:, s2, ec * 128:(ec + 1) * 128], ident[:])
                                nc.vector.tensor_copy(
                                    encT[ec][:, ss * 128:(ss + 1) * 128], ps[:])
                    for hc in range(HC):
                        kps = key_psum.tile([128, SC], F32, name="key")
                        for ec in range(EC):
                            nc.tensor.matmul(
                                kps[:], wkT[ec][:, hc * 128:(hc + 1) * 128], encT[ec][:],
                                start=(ec == 0), stop=(ec == EC - 1))
                        nc.scalar.activation(T_sb[hc][:], kps[:], ACT_F.Tanh,
                                             bias=qT[:, hc, b:b + 1])
                    sps = sc_psum.tile([1, SC], F32, name="sc")
                    for hc in range(HC):
                        nc.tensor.matmul(sps[:], weT[:, hc:hc + 1], T_sb[hc][:],
                                         start=(hc == 0), stop=(hc == HC - 1))
                    nc.vector.tensor_copy(scores[:, sc * SC:(sc + 1) * SC], sps[:])
                return scores

            def phase_b(b, scores):
                negm = small.tile([1, 1], F32, name="negm")
                nc.vector.reduce_max(negm[:], scores[:], axis=AX.X, negate=True)
                alph = apool.tile([1, S], F32, name="alph")
                ssum = small.tile([1, 1], F32, name="ssum")
                nc.scalar.activation(alph[:], scores[:], ACT_F.Exp,
                                     bias=negm[:], accum_out=ssum[:])
                rinv = small.tile([1, 1], F32, name="rinv")
                nc.vector.reciprocal(rinv[:], ssum[:])
                adram = dpool.tile([S], F32, name="adram")
                nc.sync.dma_start(adram[:], alph[:])
                aT = small.tile([128, SB], F32R, name="aT")
                nc.sync.dma_start(aT[:], adram.bitcast(F32R).rearrange("(c p) -> p c", p=128))
                return aT, rinv

            def phase_c(b, aT, rinv):
                for ecq in range(EQ):
                    cps = ctx_psum.tile([1, 512], F32, name="ctx")
                    for s128 in range(SB):
                        st2 = cstage.tile([128, 512], F32R, name="c_stage")
                        nc.sync.dma_start(
                            st2[:],
                            enc.bitcast(F32R)[b, s128 * 128:(s128 + 1) * 128,
                                              ecq * 512:(ecq + 1) * 512])
                        nc.tensor.matmul(cps[:], aT[:, s128:s128 + 1], st2[:],
                                         start=(s128 == 0), stop=(s128 == SB - 1))
                    osb = small.tile([1, 512], F32, name="osb")
                    nc.scalar.activation(osb[:], cps[:], ACT_F.Copy, scale=rinv[:])
                    nc.sync.dma_start(out[b:b + 1, ecq * 512:(ecq + 1) * 512], osb[:])

            # pipeline: A(0) B(0) A(1) B(1) C(0) A(2) B(2) C(1) A(3) B(3) C(2) C(3)
            def body():
                pend = []
                for b in range(B_L):
                    scores = phase_a(b)
                    ab = phase_b(b, scores)
                    pend.append((b, ab))
                    if b >= 1:
                        pb, (paT, prinv) = pend.pop(0)
                        phase_c(pb, paT, prinv)
                for pb, (paT, prinv) in pend:
                    phase_c(pb, paT, prinv)

            if repeat == 1:
                body()
            else:
                with tc.For_i(0, repeat, 1):
                    body()

    nc.compile()
    _CACHE[key] = nc
    return nc


def _shard_inputs(encoder_output, decoder_hidden, Wq, Wk, We):
    enc = np.ascontiguousarray(encoder_output, dtype=np.float32)
    dec = np.ascontiguousarray(decoder_hidden, dtype=np.float32).reshape(B, H)
    Wq = np.ascontiguousarray(Wq, dtype=np.float32)
    Wk = np.ascontiguousarray(Wk, dtype=np.float32)
    We = np.ascontiguousarray(We, dtype=np.float32)
    in_maps = []
    for c in range(N_CORES):
        sl = slice(c * B_L, (c + 1) * B_L)
        in_maps.append({
            "enc": enc[sl], "dec": dec[sl], "Wq": Wq, "Wk": Wk, "We": We,
        })
    return in_maps


def kernel(encoder_output, decoder_hidden, Wq, Wk, We):
    nc = _build()
    in_maps = _shard_inputs(encoder_output, decoder_hidden, Wq, Wk, We)
    res = run_bass_kernel_spmd(nc, in_maps, core_ids=list(range(N_CORES)))
    outs = [res.results[c]["out"] for c in range(N_CORES)]
    full = np.concatenate(outs, axis=0).reshape(B, 1, E).astype(np.float32)
    return full


# revision 10
# speedup vs baseline: 1.1425x; 1.1425x over previous
"""Bahdanau attention on 8 Trainium2 NeuronCores.

Data-parallel over batch: each core handles B_L = B/8 = 4 batches with all
weights replicated.  Per batch b:
  keyT[h,s]  = sum_e WkT[e,h] * encT[e,s]         (PE, fp32r)
  T[h,s]     = tanh(keyT + qT[h,b])               (ACT, bias = per-partition)
  scores[s]  = sum_h WeT[h] * T[h,s]              (PE, M=1 matmul)
  alph       = exp(scores - max)                  (ACT, accum_out = sum)
  ctx[e]     = (1/sum) * sum_s alph[s] * enc[s,e] (PE, second pass over enc)

enc arrives [S, E] in HBM; the key matmul needs it E-on-partitions, so each
[128,128] block is transposed on the PE (identity-matmul).  Four transposes
share one [128,512] PSUM bank so the PSUM->SBUF drain is a single DVE copy.
The context matmul uses enc in native layout.
"""

import sys

if "/opt/trn_rl_repo" not in sys.path:
    sys.path.insert(0, "/opt/trn_rl_repo")

import numpy as np

import concourse.bass as bass
import concourse.mybir as mybir
from concourse import bacc
from concourse.tile import TileContext
from concourse.bass_utils import run_bass_kernel_spmd
from concourse.masks import make_identity

B, S, H = 32, 2048, 1024
E = 2 * H
N_CORES = 8
B_L = B // N_CORES       # 4 batches per core
SC = 512                 # s-chunk for the key/scores phase
NSC = S // SC            # 4
SB = S // 128            # 16 s-blocks of 128
EC = E // 128            # 16 e-chunks
HC = H // 128            # 8 h-chunks
EQ = E // 512            # 4 e-quarters for the context phase

F32 = mybir.dt.float32
F32R = mybir.dt.float32r
ACT_F = mybir.ActivationFunctionType
AX = mybir.AxisListType

_CACHE = {}


def _build(repeat=1):
    key = ("nc", repeat)
    if key in _CACHE:
        return _CACHE[key]
    nc = bacc.Bacc("TRN2", target_bir_lowering=False, debug=False,
                   num_devices=N_CORES)
    enc = nc.dram_tensor("enc", [B_L, S, E], F32, kind="ExternalInput").ap()
    dec = nc.dram_tensor("dec", [B_L, H], F32, kind="ExternalInput").ap()
    wq = nc.dram_tensor("Wq", [H, H], F32, kind="ExternalInput").ap()
    wk = nc.dram_tensor("Wk", [H, E], F32, kind="ExternalInput").ap()
    we = nc.dram_tensor("We", [1, H], F32, kind="ExternalInput").ap()
    out = nc.dram_tensor("out", [B_L, E], F32, kind="ExternalOutput").ap()

    encr = enc.bitcast(F32R)
    wqr = wq.bitcast(F32R)
    wkr = wk.bitcast(F32R)

    with TileContext(nc) as tc:
        with (
            tc.tile_pool(name="const", bufs=1) as cpool,
            tc.tile_pool(name="dram", bufs=2, space="DRAM") as dpool,
            tc.tile_pool(name="enc_in", bufs=2) as epool,
            tc.tile_pool(name="encT", bufs=1) as etpool,
            tc.tile_pool(name="tpool", bufs=1) as tpool,
            tc.tile_pool(name="cstage", bufs=4) as cstage,
            tc.tile_pool(name="scores", bufs=2) as scpool,
            tc.tile_pool(name="small", bufs=2) as small,
            tc.tile_pool(name="tr_psum", bufs=2, space="PSUM") as tr_psum,
            tc.tile_pool(name="key_psum", bufs=2, space="PSUM") as key_psum,
            tc.tile_pool(name="sc_psum", bufs=2, space="PSUM") as sc_psum,
            tc.tile_pool(name="ctx_psum", bufs=2, space="PSUM") as ctx_psum,
        ):
            # ---------------- setup ----------------
            ident = cpool.tile([128, 128], F32)
            make_identity(nc, ident)

            def transpose4(dst_ap, est, j_cols, name="tr"):
                """Four [128,128] PE transposes through one [128,512] PSUM bank,
                drained by a single DVE copy into dst_ap ([128, 512])."""
                ps = tr_psum.tile([128, 512], F32R, name=name)
                for j, col in enumerate(j_cols):
                    nc.tensor.transpose(ps[:, j * 128:(j + 1) * 128],
                                        est[:, j, col * 128:(col + 1) * 128],
                                        ident[:])
                nc.vector.tensor_copy(dst_ap, ps[:])

            # WkT[ec] : [128e, H] fp32r, resident all kernel
            wkT = [cpool.tile([128, H], F32R, name=f"wkT{ec}") for ec in range(EC)]
            for hg in range(2):
                est = epool.tile([128, 4, E], F32, name="enc_in")
                for j in range(4):
                    hc = hg * 4 + j
                    nc.sync.dma_start(est[:, j, :], wk[hc * 128:(hc + 1) * 128, :])
                for ec in range(EC):
                    ps = tr_psum.tile([128, 512], F32, name="tr")
                    for j in range(4):
                        nc.tensor.transpose(ps[:, j * 128:(j + 1) * 128],
                                            est[:, j, ec * 128:(ec + 1) * 128],
                                            ident[:])
                    nc.vector.tensor_copy(wkT[ec][:, hg * 512:(hg + 1) * 512], ps[:])

            # qT[h, b] = sum_e WqT[e,h] dec[b,e]   ([128, HC, B_L] fp32 for ACT bias)
            decT = cpool.tile([128, HC, B_L], F32R)
            for b2 in range(B_L):
                nc.sync.dma_start(
                    decT[:, :, b2],
                    dec.bitcast(F32R)[b2, :].rearrange("(c p) -> p c", p=128))
            qT = cpool.tile([128, HC, B_L], F32)
            for hc in range(HC):
                est = epool.tile([128, 4, E], F32, name="enc_in")
                nc.sync.dma_start(est[:, 0, :H], wq[hc * 128:(hc + 1) * 128, :])
                qps = key_psum.tile([128, SC], F32, name="key")
                for ec in range(HC):
                    ps = tr_psum.tile([128, 512], F32, name="tr")
                    nc.tensor.transpose(ps[:, :128],
                                        est[:, 0, ec * 128:(ec + 1) * 128],
                                        ident[:])
                    blk = cstage.tile([128, 512], F32R, name="c_stage")
                    nc.vector.tensor_copy(blk[:, :128], ps[:, :128])
                    nc.tensor.matmul(qps[:, :B_L], blk[:, :128], decT[:, ec, :],
                                     start=(ec == 0), stop=(ec == HC - 1))
                nc.vector.tensor_copy(qT[:, hc, :], qps[:, :B_L])

            # WeT : [128, HC] fp32r
            weT = cpool.tile([128, HC], F32R)
            nc.sync.dma_start(weT[:], we.bitcast(F32R).rearrange("o (c p) -> p (o c)", p=128))

            # persistent working tiles
            encT = [etpool.tile([128, SC], F32R, name=f"encT{ec}") for ec in range(EC)]
            T_sb = [tpool.tile([128, SC], F32R, name=f"T{hc}") for hc in range(HC)]

            def phase_a(b):
                scores = scpool.tile([1, S], F32, name="scores")
                for sc in range(NSC):
                    est = epool.tile([128, 4, E], F32, name="enc_in")
                    for ss in range(4):
                        r0 = sc * SC + ss * 128
                        nc.sync.dma_start(est[:, ss, :], enc[b, r0:r0 + 128, :])
                    for ec in range(EC):
                        ps = tr_psum.tile([128, 512], F32, name="tr")
                        for ss in range(4):
                            nc.tensor.transpose(ps[:, ss * 128:(ss + 1) * 128],
                                                est[:, ss, ec * 128:(ec + 1) * 128],
                                                ident[:])
                        nc.vector.tensor_copy(encT[ec][:], ps[:])
                    for hc in range(HC):
                        kps = key_psum.tile([128, SC], F32, name="key")
                        for ec in range(EC):
                            nc.tensor.matmul(
                                kps[:], wkT[ec][:, hc * 128:(hc + 1) * 128], encT[ec][:],
                                start=(ec == 0), stop=(ec == EC - 1))
                        nc.scalar.activation(T_sb[hc][:], kps[:], ACT_F.Tanh,
                                             bias=qT[:, hc, b:b + 1])
                    sps = sc_psum.tile([1, SC], F32, name="sc")
                    for hc in range(HC):
                        nc.tensor.matmul(sps[:], weT[:, hc:hc + 1], T_sb[hc][:],
                                         start=(hc == 0), stop=(hc == HC - 1))
                    nc.vector.tensor_copy(scores[:, sc * SC:(sc + 1) * SC], sps[:])
                return scores

            def phase_b(b, scores):
                negm = small.tile([1, 1], F32, name="negm")
                nc.vector.reduce_max(negm[:], scores[:], axis=AX.X, negate=True)
                ssum = small.tile([1, 1], F32, name="ssum")
                alph = scpool.tile([1, S], F32, name="scores")
                nc.scalar.activation(alph[:], scores[:], ACT_F.Exp,
                                     bias=negm[:], accum_out=ssum[:])
                rinv = small.tile([1, 1], F32, name="rinv")
                nc.vector.reciprocal(rinv[:], ssum[:])
                adram = dpool.tile([S], F32, name="adram")
                nc.sync.dma_start(adram[:], alph[:])
                aT = small.tile([128, SB], F32R, name="aT")
                nc.sync.dma_start(aT[:], adram.bitcast(F32R).rearrange("(c p) -> p c", p=128))
                return aT, rinv

            def phase_c(b, aT, rinv):
                for ecq in range(EQ):
                    cps = ctx_psum.tile([1, 512], F32, name="ctx")
                    for s128 in range(SB):
                        st2 = cstage.tile([128, 512], F32R, name="c_stage")
                        nc.sync.dma_start(
                            st2[:],
                            encr[b, s128 * 128:(s128 + 1) * 128,
                                 ecq * 512:(ecq + 1) * 512])
                        nc.tensor.matmul(cps[:], aT[:, s128:s128 + 1], st2[:],
                                         start=(s128 == 0), stop=(s128 == SB - 1))
                    osb = small.tile([1, 512], F32, name="osb")
                    nc.scalar.activation(osb[:], cps[:], ACT_F.Copy, scale=rinv[:])
                    nc.sync.dma_start(out[b:b + 1, ecq * 512:(ecq + 1) * 512], osb[:])

            # pipeline: A(0) B(0) A(1) B(1) C(0) A(2) B(2) C(1) A(3) B(3) C(2) C(3)
            def body():
                pend = []
                for b in range(B_L):
                    scores = phase_a(b)
                    ab = phase_b(b, scores)
                    pend.append((b, ab))
                    if b >= 1:
                        pb, (paT, prinv) = pend.pop(0)
                        phase_c(pb, paT, prinv)
                for pb, (paT, prinv) in pend:
                    phase_c(pb, paT, prinv)

            if repeat == 1:
                body()
            else:
                with tc.For_i(0, repeat, 1):
                    body()

    nc.compile()
    _CACHE[key] = nc
    return nc


def _shard_inputs(encoder_output, decoder_hidden, Wq, Wk, We):
    enc = np.ascontiguousarray(encoder_output, dtype=np.float32)
    dec = np.ascontiguousarray(decoder_hidden, dtype=np.float32).reshape(B, H)
    Wq = np.ascontiguousarray(Wq, dtype=np.float32)
    Wk = np.ascontiguousarray(Wk, dtype=np.float32)
    We = np.ascontiguousarray(We, dtype=np.float32)
    in_maps = []
    for c in range(N_CORES):
        sl = slice(c * B_L, (c + 1) * B_L)
        in_maps.append({
            "enc": enc[sl], "dec": dec[sl], "Wq": Wq, "Wk": Wk, "We": We,
        })
    return in_maps


def kernel(encoder_output, decoder_hidden, Wq, Wk, We):
    nc = _build()
    in_maps = _shard_inputs(encoder_output, decoder_hidden, Wq, Wk, We)
    res = run_bass_kernel_spmd(nc, in_maps, core_ids=list(range(N_CORES)))
    outs = [res.results[c]["out"] for c in range(N_CORES)]
    full = np.concatenate(outs, axis=0).reshape(B, 1, E).astype(np.float32)
    return full
